# revision 25
# baseline (speedup 1.0000x reference)
"""Trainium2 Bass kernel for nn_MultiHeadAttention_61546881352366.

The reference module's observable output is NOT attention: the attention
result is dead code in the original torch module.  The output is

    out = fc0(concat_h(v @ Wv_h^T)) = (v @ Wcat^T) @ W0^T + b0

with Wcat = Wv.reshape(H*D, C).  Two chained linear maps fuse into one:

    out = v @ (W0 @ Wcat)^T + b0 = v @ WcT + b0,   WcT = (W0 @ Wcat)^T

so the device work is a single [B*T, C] @ [C, C] matmul.  k and q are
unused.  The bias add and the final upcast to fp32 happen on the host.

Sharding: data-parallel over batch (B == 8 == n_cores); each core computes
one batch element's [2048, 1024] @ [1024, 1024] product in bf16 (fp32 PSUM
accumulate; rel err ~3e-3 incl. the bf16 output rounding).  Weights are
replicated (2 MiB/core).

Device kernel (per core):
  - inputs laid out on the host so every DMA moves contiguous
    per-partition lines:
      vp [128, 16, 8, 128] bf16   vp[p,m,k,t] = v[m*128+t, k*128+p]
      wp [128, 8, 1024]   bf16    wp[p,k,j]   = WcT[k*128+p, j]
  - 256 matmuls of [128x128] @ [128x512] bf16 at the warm PE rate
    (~216 ns each)
  - fill phase ordered to match DMA arrival (small first chunks, w on
    the scalar queue / v on the sync queue in parallel) so the PE never
    starves while inputs stream in; a few warmup matmuls ramp the HAM
    clock gate before the first chunks land
  - per (row tile, column half): one [128,512] PSUM bank, one vector
    copy fp32->bf16 into SBUF, one 128 KiB output DMA on the scalar
    queue; j-outer order so each bank drains 8 matmuls before the row
    tile finishes -- only one half-drain trails the final matmul
"""

import numpy as np

import concourse.bacc as bacc
import concourse.mybir as mybir
from concourse.tile import TileContext
from concourse.bass_utils import run_bass_kernel_spmd

B, T, C = 8, 2048, 1024
H, D = 16, 64
P = 128
KT = C // P   # 8 contraction tiles
MT = T // P   # 16 row tiles per core
NF = 512      # matmul moving free dim (= one PSUM bank of fp32)
NJ = C // NF  # 2 output column halves

_FP32 = mybir.dt.float32
_BF16 = mybir.dt.bfloat16

N_WARMUP = 8   # dummy matmuls sized to keep the PE busy from the end of the
               # engine preamble (~7.8us) until the first input chunks land
               # (~11.2us) -- the HAM clock gate then unthrottles BEFORE the
               # first real matmul, so the whole real stream runs at 2.4 GHz


def _build():
    nc = bacc.Bacc()
    vp = nc.dram_tensor("vp", [P, MT, KT, P], _BF16, kind="ExternalInput")
    wp = nc.dram_tensor("wp", [P, KT, C], _BF16, kind="ExternalInput")
    out = nc.dram_tensor("out", [T, C], _BF16, kind="ExternalOutput")

    with TileContext(nc) as tc:
        with (
            tc.tile_pool(name="wpool", bufs=1) as wpool,
            tc.tile_pool(name="vpool", bufs=1) as vpool,
            tc.tile_pool(name="spool", bufs=1) as spool,
            tc.tile_pool(name="opool", bufs=8) as opool,
            tc.tile_pool(name="pspool", bufs=8, space="PSUM") as pspool,
        ):
            # PE warmup: dependency-free matmuls on a memset tile so the HAM
            # clock gate ramps to 2.4 GHz while the first DMAs are in flight.
            # gpsimd does the memset -- it is free earliest in the preamble.
            scratch = spool.tile([P, NF], _BF16, name="scratch", tag="scratch")
            nc.gpsimd.memset(scratch, 0.0)
            ps_w = pspool.tile([P, NF], _FP32, name="ps_w", tag="ps")
            for _ in range(N_WARMUP):
                nc.tensor.matmul(
                    ps_w, lhsT=scratch[:, :P], rhs=scratch, start=True, stop=True
                )

            # Input DMAs across the two HW-DGE queues (sync, scalar), ordered
            # so the fill phase's dependencies land in consumption order.
            # The first chunks are small so the very first matmuls' inputs
            # arrive as early as possible.
            w_sb = {}           # (k, j) -> (tile, row index, column offset)
            v_sb = [None] * MT  # per m-tile: (tile, index within group)

            def dma_w(ks, eng, jhalf=None):
                # jhalf None: full C columns; 0/1: one 512-wide half
                lo = 0 if jhalf is None else jhalf * NF
                hi = C if jhalf is None else (jhalf + 1) * NF
                w_c = wpool.tile(
                    [P, len(ks), hi - lo],
                    _BF16,
                    name=f"w_{ks[0]}_{lo}",
                    tag=f"w_{ks[0]}_{lo}",
                )
                eng.dma_start(out=w_c, in_=wp[:, ks[0] : ks[-1] + 1, lo:hi])
                for i, k in enumerate(ks):
                    for j in range(NJ):
                        if jhalf is None:
                            w_sb[(k, j)] = (w_c, i, j * NF)
                        elif j == jhalf:
                            w_sb[(k, j)] = (w_c, i, 0)

            def dma_v(lo, hi, eng):
                vt = vpool.tile(
                    [P, hi - lo, KT, P], _BF16, name=f"v_{lo}", tag=f"v_{lo}"
                )
                eng.dma_start(out=vt, in_=vp[:, lo:hi, :, :])
                for m in range(lo, hi):
                    v_sb[m] = (vt, m - lo)

            # Round-robin the chunks across three queues (sync + scalar
            # HW-DGE, gpsimd SW-DGE) in consumption order -- which queue
            # starts first / runs faster varies run to run, so no queue may
            # own the whole critical path.  gpsimd's SW-DGE has the highest
            # first-byte latency, so it gets the less urgent v chunks.
            chunks = [
                lambda e: dma_w([0, 1], e),    # w k01     (512K)
                lambda e: dma_v(0, 1, e),      # v m0      (256K)
                lambda e: dma_v(1, 2, e),      # v m1      (256K)
                lambda e: dma_w([2, 3], e),    # w k23     (512K)
                lambda e: dma_v(2, 3, e),      # v m2      (256K)
                lambda e: dma_v(3, 4, e),      # v m3      (256K)
                lambda e: dma_w([4, 5], e),    # w k45     (512K)
                lambda e: dma_w([6, 7], e),    # w k67     (512K)
                lambda e: dma_v(4, 8, e),      # v m4-7    (1M)
                lambda e: dma_v(8, 12, e),     # v m8-11   (1M)
                lambda e: dma_v(12, 16, e),    # v m12-15  (1M)
            ]
            engs = [nc.sync, nc.scalar, nc.gpsimd]
            for i, ch in enumerate(chunks):
                ch(engs[i % 3])

            def mm(ps_mj, m, k, j):
                vt, s = v_sb[m]
                wt, i, off = w_sb[(k, j)]
                nc.tensor.matmul(
                    ps_mj,
                    lhsT=vt[:, s, k, :],
                    rhs=wt[:, i, off : off + NF],
                    start=(k == 0),
                    stop=(k == KT - 1),
                )

            ps = {}
            ob = {}

            def new_ps(m):
                ps[m] = [
                    pspool.tile([P, NF], _FP32, name=f"ps_{m}_{j}", tag="ps")
                    for j in range(NJ)
                ]

            def drain(m, j):
                if m not in ob:
                    ob[m] = opool.tile([P, C], _BF16, name=f"ob_{m}", tag="ob")
                rows = slice(m * P, (m + 1) * P)
                if m == MT - 1 and j == NJ - 1:
                    # Final drain: two parallel quarter copies (vector +
                    # gpsimd) and two parallel output DMAs (scalar + sync)
                    # so only ~a quarter drain trails the last matmul.
                    h = NF // 2
                    lo, mid, hi = j * NF, j * NF + h, (j + 1) * NF
                    nc.vector.tensor_copy(ob[m][:, lo:mid], ps[m][j][:, :h])
                    nc.scalar.copy(ob[m][:, mid:hi], ps[m][j][:, h:])
                    nc.scalar.dma_start(out=out[rows, lo:mid], in_=ob[m][:, lo:mid])
                    nc.scalar.dma_start(out=out[rows, mid:hi], in_=ob[m][:, mid:hi])
                    return
                sl = slice(j * NF, (j + 1) * NF)
                nc.vector.tensor_copy(ob[m][:, sl], ps[m][j])
                nc.scalar.dma_start(out=out[rows, sl], in_=ob[m][:, sl])

            # Fill phase (m0-3): blocks ordered to match DMA chunk arrival.
            # Each entry is (k-list, m-list, j-list), emitted m/k/j nested.
            for m in range(4):
                new_ps(m)
            fill = [
                ([0, 1], [0], [0, 1]),
                ([0, 1], [1], [0, 1]),
                ([0, 1], [2], [0, 1]),
                ([0, 1], [3], [0, 1]),
                ([2, 3], [0, 1], [0, 1]),
                ([2, 3], [2, 3], [0, 1]),
                ([4, 5], [0, 1, 2, 3], [0, 1]),
            ]
            for ks, ms, js in fill:
                for m in ms:
                    for k in ks:
                        for j in js:
                            mm(ps[m][j], m, k, j)
            # last fill chunk: k6-7, j-outer per m so each bank drains early
            for m in range(4):
                for j in range(NJ):
                    for k in (6, 7):
                        mm(ps[m][j], m, k, j)
                    drain(m, j)

            # Steady phase (m4-15): j-outer, k-inner; each column half
            # drains as soon as its 8 matmuls finish.
            for m in range(4, MT):
                new_ps(m)
                for j in range(NJ):
                    for k in range(KT):
                        mm(ps[m][j], m, k, j)
                    drain(m, j)
    nc.compile()
    return nc


_nc_cache = None


def _get_nc():
    global _nc_cache
    if _nc_cache is None:
        _nc_cache = _build()
    return _nc_cache


def prepare_inputs(inputs):
    """Host-side prep shared by kernel() and the timing harness.

    Returns (in_maps, b0): per-core device inputs and the bias to add on
    the host after the gather.
    """
    import ml_dtypes

    v = np.ascontiguousarray(np.asarray(inputs["v"], dtype=np.float32))
    Wv = np.asarray(inputs["Wv"], dtype=np.float32)
    W0 = np.asarray(inputs["W0"], dtype=np.float32)
    b0 = np.asarray(inputs["b0"], dtype=np.float32)

    # Fuse the two linear layers on the host: WcT = (W0 @ Wcat)^T  [C_in, C_out]
    Wc = W0 @ Wv.reshape(H * D, C)
    # wp[p, k, j] = WcT[k*128+p, j]
    wp = np.ascontiguousarray(
        Wc.T.reshape(KT, P, C).transpose(1, 0, 2).astype(ml_dtypes.bfloat16)
    )
    # vp[b][p, m, k, t] = v[b, m*128+t, k*128+p]
    vp = np.ascontiguousarray(
        v.reshape(B, MT, P, KT, P).transpose(0, 4, 1, 3, 2).astype(ml_dtypes.bfloat16)
    )
    return [{"vp": vp[i], "wp": wp} for i in range(B)], b0


def kernel(**inputs):
    in_maps, b0 = prepare_inputs(inputs)
    nc = _get_nc()
    res = run_bass_kernel_spmd(nc, in_maps, core_ids=list(range(B)))
    out = np.stack([res.results[i]["out"] for i in range(B)], axis=0)
    return out.astype(np.float32) + b0


# revision 29
# speedup vs baseline: 1.0658x; 1.0658x over previous
"""Trainium2 Bass kernel for nn_MultiHeadAttention_61546881352366.

The reference module's observable output is NOT attention: the attention
result is dead code in the original torch module.  The output is

    out = fc0(concat_h(v @ Wv_h^T)) = (v @ Wcat^T) @ W0^T + b0

with Wcat = Wv.reshape(H*D, C).  Two chained linear maps fuse into one:

    out = v @ (W0 @ Wcat)^T + b0 = v @ WcT + b0,   WcT = (W0 @ Wcat)^T

so the device work is a single [B*T, C] @ [C, C] matmul.  k and q are
unused.  The bias add and the final upcast to fp32 happen on the host.

Sharding: data-parallel over batch (B == 8 == n_cores); each core computes
one batch element's [2048, 1024] @ [1024, 1024] product in bf16 (fp32 PSUM
accumulate; rel err ~3e-3 incl. the bf16 output rounding).  Weights are
replicated (2 MiB/core).

Device kernel (per core):
  - inputs laid out on the host so every DMA moves contiguous
    per-partition lines:
      vp [128, 16, 8, 128] bf16   vp[p,m,k,t] = v[m*128+t, k*128+p]
      wp [128, 8, 1024]   bf16    wp[p,k,j]   = WcT[k*128+p, j]
  - 256 matmuls of [128x128] @ [128x512] bf16 at the warm PE rate
    (~216 ns each)
  - fill phase ordered to match DMA arrival (small first chunks, w on
    the scalar queue / v on the sync queue in parallel) so the PE never
    starves while inputs stream in; a few warmup matmuls ramp the HAM
    clock gate before the first chunks land
  - per (row tile, column half): one [128,512] PSUM bank, one vector
    copy fp32->bf16 into SBUF, one 128 KiB output DMA on the scalar
    queue; j-outer order so each bank drains 8 matmuls before the row
    tile finishes -- only one half-drain trails the final matmul
"""

import numpy as np

import concourse.bacc as bacc
import concourse.mybir as mybir
from concourse.tile import TileContext
from concourse.bass_utils import run_bass_kernel_spmd

B, T, C = 8, 2048, 1024
H, D = 16, 64
P = 128
KT = C // P   # 8 contraction tiles
MT = T // P   # 16 row tiles per core
NF = 512      # matmul moving free dim (= one PSUM bank of fp32)
NJ = C // NF  # 2 output column halves

_FP32 = mybir.dt.float32
_BF16 = mybir.dt.bfloat16

N_WARMUP = 11  # dummy matmuls sized to keep the PE busy from the end of the
               # engine preamble (~7.8us) until the first input chunks land
               # (~12.6us) -- the HAM clock gate then unthrottles BEFORE the
               # first real matmul, so the whole real stream runs at 2.4 GHz


def _build():
    nc = bacc.Bacc()
    vp = nc.dram_tensor("vp", [P, MT, KT, P], _BF16, kind="ExternalInput")
    wp = nc.dram_tensor("wp", [P, KT, C], _BF16, kind="ExternalInput")
    out = nc.dram_tensor("out", [T, C], _BF16, kind="ExternalOutput")

    with TileContext(nc) as tc:
        with (
            tc.tile_pool(name="wpool", bufs=1) as wpool,
            tc.tile_pool(name="vpool", bufs=1) as vpool,
            tc.tile_pool(name="spool", bufs=1) as spool,
            tc.tile_pool(name="opool", bufs=8) as opool,
            tc.tile_pool(name="pspool", bufs=8, space="PSUM") as pspool,
        ):
            # PE warmup: dependency-free matmuls on a memset tile so the HAM
            # clock gate ramps to 2.4 GHz while the first DMAs are in flight.
            # gpsimd does the memset -- it is free earliest in the preamble.
            scratch = spool.tile([P, NF], _BF16, name="scratch", tag="scratch")
            nc.gpsimd.memset(scratch, 0.0)
            ps_w = pspool.tile([P, NF], _FP32, name="ps_w", tag="ps")
            for _ in range(N_WARMUP):
                nc.tensor.matmul(
                    ps_w, lhsT=scratch[:, :P], rhs=scratch, start=True, stop=True
                )

            # Input DMAs across the two HW-DGE queues (sync, scalar), ordered
            # so the fill phase's dependencies land in consumption order.
            # The first chunks are small so the very first matmuls' inputs
            # arrive as early as possible.
            w_sb = {}           # (k, j) -> (tile, row index, column offset)
            v_sb = [None] * MT  # per m-tile: (tile, index within group)

            def dma_w(ks, eng, jhalf=None):
                # jhalf None: full C columns; 0/1: one 512-wide half
                lo = 0 if jhalf is None else jhalf * NF
                hi = C if jhalf is None else (jhalf + 1) * NF
                w_c = wpool.tile(
                    [P, len(ks), hi - lo],
                    _BF16,
                    name=f"w_{ks[0]}_{lo}",
                    tag=f"w_{ks[0]}_{lo}",
                )
                eng.dma_start(out=w_c, in_=wp[:, ks[0] : ks[-1] + 1, lo:hi])
                for i, k in enumerate(ks):
                    for j in range(NJ):
                        if jhalf is None:
                            w_sb[(k, j)] = (w_c, i, j * NF)
                        elif j == jhalf:
                            w_sb[(k, j)] = (w_c, i, 0)

            def dma_v(lo, hi, eng):
                vt = vpool.tile(
                    [P, hi - lo, KT, P], _BF16, name=f"v_{lo}", tag=f"v_{lo}"
                )
                eng.dma_start(out=vt, in_=vp[:, lo:hi, :, :])
                for m in range(lo, hi):
                    v_sb[m] = (vt, m - lo)

            # Round-robin the chunks across the two HW-DGE queues (sync,
            # scalar) in consumption order -- which queue starts first /
            # runs faster varies run to run, so neither queue may own the
            # whole critical path.
            chunks = [
                lambda e: dma_w([0, 1], e),    # w k01     (512K)
                lambda e: dma_v(0, 1, e),      # v m0      (256K)
                lambda e: dma_v(1, 2, e),      # v m1      (256K)
                lambda e: dma_w([2, 3], e),    # w k23     (512K)
                lambda e: dma_v(2, 3, e),      # v m2      (256K)
                lambda e: dma_v(3, 4, e),      # v m3      (256K)
                lambda e: dma_w([4, 5], e),    # w k45     (512K)
                lambda e: dma_w([6, 7], e),    # w k67     (512K)
                lambda e: dma_v(4, 8, e),      # v m4-7    (1M)
                lambda e: dma_v(8, 12, e),     # v m8-11   (1M)
                lambda e: dma_v(12, 16, e),    # v m12-15  (1M)
            ]
            for i, ch in enumerate(chunks):
                ch(nc.sync if i % 2 == 0 else nc.scalar)

            def mm(ps_mj, m, k, j):
                vt, s = v_sb[m]
                wt, i, off = w_sb[(k, j)]
                nc.tensor.matmul(
                    ps_mj,
                    lhsT=vt[:, s, k, :],
                    rhs=wt[:, i, off : off + NF],
                    start=(k == 0),
                    stop=(k == KT - 1),
                )

            ps = {}
            ob = {}

            def new_ps(m):
                ps[m] = [
                    pspool.tile([P, NF], _FP32, name=f"ps_{m}_{j}", tag="ps")
                    for j in range(NJ)
                ]

            def drain(m, j):
                if m not in ob:
                    ob[m] = opool.tile([P, C], _BF16, name=f"ob_{m}", tag="ob")
                rows = slice(m * P, (m + 1) * P)
                if m == MT - 1 and j == NJ - 1:
                    # Final drain: two parallel quarter copies (vector +
                    # gpsimd) and two parallel output DMAs (scalar + sync)
                    # so only ~a quarter drain trails the last matmul.
                    h = NF // 2
                    lo, mid, hi = j * NF, j * NF + h, (j + 1) * NF
                    nc.vector.tensor_copy(ob[m][:, lo:mid], ps[m][j][:, :h])
                    nc.scalar.copy(ob[m][:, mid:hi], ps[m][j][:, h:])
                    nc.scalar.dma_start(out=out[rows, lo:mid], in_=ob[m][:, lo:mid])
                    nc.scalar.dma_start(out=out[rows, mid:hi], in_=ob[m][:, mid:hi])
                    return
                sl = slice(j * NF, (j + 1) * NF)
                nc.vector.tensor_copy(ob[m][:, sl], ps[m][j])
                nc.scalar.dma_start(out=out[rows, sl], in_=ob[m][:, sl])

            # Fill phase (m0-3): blocks ordered to match DMA chunk arrival.
            # Each entry is (k-list, m-list, j-list), emitted m/k/j nested.
            for m in range(4):
                new_ps(m)
            fill = [
                ([0, 1], [0], [0, 1]),
                ([0, 1], [1], [0, 1]),
                ([2, 3], [0, 1], [0, 1]),
                ([0, 1], [2], [0, 1]),
                ([0, 1], [3], [0, 1]),
                ([2, 3], [2, 3], [0, 1]),
                ([4, 5], [0, 1, 2, 3], [0, 1]),
            ]
            for ks, ms, js in fill:
                for m in ms:
                    for k in ks:
                        for j in js:
                            mm(ps[m][j], m, k, j)
            # last fill chunk: k6-7, j-outer per m so each bank drains early
            for m in range(4):
                for j in range(NJ):
                    for k in (6, 7):
                        mm(ps[m][j], m, k, j)
                    drain(m, j)

            # Steady phase (m4-15): j-outer, k-inner; each column half
            # drains as soon as its 8 matmuls finish.
            for m in range(4, MT):
                new_ps(m)
                for j in range(NJ):
                    for k in range(KT):
                        mm(ps[m][j], m, k, j)
                    drain(m, j)
    nc.compile()
    return nc


_nc_cache = None


def _get_nc():
    global _nc_cache
    if _nc_cache is None:
        _nc_cache = _build()
    return _nc_cache


def prepare_inputs(inputs):
    """Host-side prep shared by kernel() and the timing harness.

    Returns (in_maps, b0): per-core device inputs and the bias to add on
    the host after the gather.
    """
    import ml_dtypes

    v = np.ascontiguousarray(np.asarray(inputs["v"], dtype=np.float32))
    Wv = np.asarray(inputs["Wv"], dtype=np.float32)
    W0 = np.asarray(inputs["W0"], dtype=np.float32)
    b0 = np.asarray(inputs["b0"], dtype=np.float32)

    # Fuse the two linear layers on the host: WcT = (W0 @ Wcat)^T  [C_in, C_out]
    Wc = W0 @ Wv.reshape(H * D, C)
    # wp[p, k, j] = WcT[k*128+p, j]
    wp = np.ascontiguousarray(
        Wc.T.reshape(KT, P, C).transpose(1, 0, 2).astype(ml_dtypes.bfloat16)
    )
    # vp[b][p, m, k, t] = v[b, m*128+t, k*128+p]
    vp = np.ascontiguousarray(
        v.reshape(B, MT, P, KT, P).transpose(0, 4, 1, 3, 2).astype(ml_dtypes.bfloat16)
    )
    return [{"vp": vp[i], "wp": wp} for i in range(B)], b0


def kernel(**inputs):
    in_maps, b0 = prepare_inputs(inputs)
    nc = _get_nc()
    res = run_bass_kernel_spmd(nc, in_maps, core_ids=list(range(B)))
    out = np.stack([res.results[i]["out"] for i in range(B)], axis=0)
    return out.astype(np.float32) + b0


# revision 32
# speedup vs baseline: 1.0782x; 1.0116x over previous
"""Trainium2 Bass kernel for nn_MultiHeadAttention_61546881352366.

The reference module's observable output is NOT attention: the attention
result is dead code in the original torch module.  The output is

    out = fc0(concat_h(v @ Wv_h^T)) = (v @ Wcat^T) @ W0^T + b0

with Wcat = Wv.reshape(H*D, C).  Two chained linear maps fuse into one:

    out = v @ (W0 @ Wcat)^T + b0 = v @ WcT + b0,   WcT = (W0 @ Wcat)^T

so the device work is a single [B*T, C] @ [C, C] matmul.  k and q are
unused.  The bias add and the final upcast to fp32 happen on the host.

Sharding: data-parallel over batch (B == 8 == n_cores); each core computes
one batch element's [2048, 1024] @ [1024, 1024] product in bf16 (fp32 PSUM
accumulate; rel err ~3e-3 incl. the bf16 output rounding).  Weights are
replicated (2 MiB/core).

Device kernel (per core):
  - inputs laid out on the host so every DMA moves contiguous
    per-partition lines:
      vp [128, 16, 8, 128] bf16   vp[p,m,k,t] = v[m*128+t, k*128+p]
      wp [128, 8, 1024]   bf16    wp[p,k,j]   = WcT[k*128+p, j]
  - 256 matmuls of [128x128] @ [128x512] bf16 at the warm PE rate
    (~216 ns each)
  - fill phase ordered to match DMA arrival (small first chunks, w on
    the scalar queue / v on the sync queue in parallel) so the PE never
    starves while inputs stream in; a few warmup matmuls ramp the HAM
    clock gate before the first chunks land
  - per (row tile, column half): one [128,512] PSUM bank, one vector
    copy fp32->bf16 into SBUF, one 128 KiB output DMA on the scalar
    queue; j-outer order so each bank drains 8 matmuls before the row
    tile finishes -- only one half-drain trails the final matmul
"""

import numpy as np

import concourse.bacc as bacc
import concourse.mybir as mybir
from concourse.tile import TileContext
from concourse.bass_utils import run_bass_kernel_spmd

B, T, C = 8, 2048, 1024
H, D = 16, 64
P = 128
KT = C // P   # 8 contraction tiles
MT = T // P   # 16 row tiles per core
NF = 512      # matmul moving free dim (= one PSUM bank of fp32)
NJ = C // NF  # 2 output column halves

_FP32 = mybir.dt.float32
_BF16 = mybir.dt.bfloat16

N_WARMUP = 11  # dummy matmuls sized to keep the PE busy from the end of the
               # engine preamble (~7.8us) until the first input chunks land
               # (~12.6us) -- the HAM clock gate then unthrottles BEFORE the
               # first real matmul, so the whole real stream runs at 2.4 GHz


def _build():
    nc = bacc.Bacc()
    vp = nc.dram_tensor("vp", [P, MT, KT, P], _BF16, kind="ExternalInput")
    wp = nc.dram_tensor("wp", [P, KT, C], _BF16, kind="ExternalInput")
    out = nc.dram_tensor("out", [T, C], _BF16, kind="ExternalOutput")

    with TileContext(nc) as tc:
        with (
            tc.tile_pool(name="wpool", bufs=1) as wpool,
            tc.tile_pool(name="vpool", bufs=1) as vpool,
            tc.tile_pool(name="spool", bufs=1) as spool,
            tc.tile_pool(name="opool", bufs=8) as opool,
            tc.tile_pool(name="pspool", bufs=8, space="PSUM") as pspool,
        ):
            # PE warmup: dependency-free matmuls on a memset tile so the HAM
            # clock gate ramps to 2.4 GHz while the first DMAs are in flight.
            # gpsimd does the memset -- it is free earliest in the preamble.
            scratch = spool.tile([P, NF], _BF16, name="scratch", tag="scratch")
            nc.gpsimd.memset(scratch, 0.0)
            ps_w = pspool.tile([P, NF], _FP32, name="ps_w", tag="ps")
            for _ in range(N_WARMUP):
                nc.tensor.matmul(
                    ps_w, lhsT=scratch[:, :P], rhs=scratch, start=True, stop=True
                )

            # Input DMAs across the two HW-DGE queues (sync, scalar), ordered
            # so the fill phase's dependencies land in consumption order.
            # The first chunks are small so the very first matmuls' inputs
            # arrive as early as possible.
            w_sb = {}           # (k, j) -> (tile, row index, column offset)
            v_sb = [None] * MT  # per m-tile: (tile, index within group)

            def dma_w(ks, eng, jhalf=None):
                # jhalf None: full C columns; 0/1: one 512-wide half
                lo = 0 if jhalf is None else jhalf * NF
                hi = C if jhalf is None else (jhalf + 1) * NF
                w_c = wpool.tile(
                    [P, len(ks), hi - lo],
                    _BF16,
                    name=f"w_{ks[0]}_{lo}",
                    tag=f"w_{ks[0]}_{lo}",
                )
                eng.dma_start(out=w_c, in_=wp[:, ks[0] : ks[-1] + 1, lo:hi])
                for i, k in enumerate(ks):
                    for j in range(NJ):
                        if jhalf is None:
                            w_sb[(k, j)] = (w_c, i, j * NF)
                        elif j == jhalf:
                            w_sb[(k, j)] = (w_c, i, 0)

            def dma_v(lo, hi, eng):
                vt = vpool.tile(
                    [P, hi - lo, KT, P], _BF16, name=f"v_{lo}", tag=f"v_{lo}"
                )
                eng.dma_start(out=vt, in_=vp[:, lo:hi, :, :])
                for m in range(lo, hi):
                    v_sb[m] = (vt, m - lo)

            # Round-robin the chunks across the two HW-DGE queues (sync,
            # scalar) in consumption order -- which queue starts first /
            # runs faster varies run to run, so neither queue may own the
            # whole critical path.
            chunks = [
                lambda e: dma_w([0, 1], e),    # w k01     (512K)
                lambda e: dma_v(0, 1, e),      # v m0      (256K)
                lambda e: dma_v(1, 2, e),      # v m1      (256K)
                lambda e: dma_w([2, 3], e),    # w k23     (512K)
                lambda e: dma_v(2, 3, e),      # v m2      (256K)
                lambda e: dma_v(3, 4, e),      # v m3      (256K)
                lambda e: dma_w([4, 5], e),    # w k45     (512K)
                lambda e: dma_w([6, 7], e),    # w k67     (512K)
                lambda e: dma_v(4, 8, e),      # v m4-7    (1M)
                lambda e: dma_v(8, 12, e),     # v m8-11   (1M)
                lambda e: dma_v(12, 16, e),    # v m12-15  (1M)
            ]
            for i, ch in enumerate(chunks):
                ch(nc.sync if i % 2 == 0 else nc.scalar)

            def mm(ps_mj, m, k, j, colo=0, width=NF):
                vt, s = v_sb[m]
                wt, i, off = w_sb[(k, j)]
                nc.tensor.matmul(
                    ps_mj,
                    lhsT=vt[:, s, k, :],
                    rhs=wt[:, i, off + colo : off + colo + width],
                    start=(k == 0),
                    stop=(k == KT - 1),
                )

            ps = {}
            ob = {}

            def new_ps(m):
                ps[m] = [
                    pspool.tile([P, NF], _FP32, name=f"ps_{m}_{j}", tag="ps")
                    for j in range(NJ)
                ]

            def drain(m, j):
                if m not in ob:
                    ob[m] = opool.tile([P, C], _BF16, name=f"ob_{m}", tag="ob")
                rows = slice(m * P, (m + 1) * P)
                sl = slice(j * NF, (j + 1) * NF)
                nc.vector.tensor_copy(ob[m][:, sl], ps[m][j])
                nc.scalar.dma_start(out=out[rows, sl], in_=ob[m][:, sl])

            def drain_part(m, base, width, src):
                rows = slice(m * P, (m + 1) * P)
                sl = slice(base, base + width)
                nc.vector.tensor_copy(ob[m][:, sl], src)
                nc.scalar.dma_start(out=out[rows, sl], in_=ob[m][:, sl])

            # Fill phase (m0-3): blocks ordered to match DMA chunk arrival.
            # Each entry is (k-list, m-list, j-list), emitted m/k/j nested.
            for m in range(4):
                new_ps(m)
            fill = [
                ([0, 1], [0], [0, 1]),
                ([0, 1], [1], [0, 1]),
                ([2, 3], [0, 1], [0, 1]),
                ([0, 1], [2], [0, 1]),
                ([0, 1], [3], [0, 1]),
                ([2, 3], [2, 3], [0, 1]),
                ([4, 5], [0, 1, 2, 3], [0, 1]),
            ]
            for ks, ms, js in fill:
                for m in ms:
                    for k in ks:
                        for j in js:
                            mm(ps[m][j], m, k, j)
            # last fill chunk: k6-7, j-outer per m so each bank drains early
            for m in range(4):
                for j in range(NJ):
                    for k in (6, 7):
                        mm(ps[m][j], m, k, j)
                    drain(m, j)

            # Steady phase (m4-15): j-outer, k-inner; each column half
            # drains as soon as its 8 matmuls finish.
            for m in range(4, MT - 1):
                new_ps(m)
                for j in range(NJ):
                    for k in range(KT):
                        mm(ps[m][j], m, k, j)
                    drain(m, j)

            # Last row tile: j1 runs as two 256-wide accumulation groups
            # (same total matmul time -- N=256 gaps are half of N=512) so
            # only a quarter-width drain trails the very last matmul.
            m = MT - 1
            new_ps(m)
            ob[m] = opool.tile([P, C], _BF16, name=f"ob_{m}", tag="ob")
            for k in range(KT):
                mm(ps[m][0], m, k, 0)
            drain(m, 0)
            psq = [
                pspool.tile([P, NF // 2], _FP32, name=f"ps_{m}_1{g}", tag="ps")
                for g in range(2)
            ]
            for g in range(2):
                for k in range(KT):
                    mm(psq[g], m, k, 1, colo=g * (NF // 2), width=NF // 2)
                drain_part(m, NF + g * (NF // 2), NF // 2, psq[g])
    nc.compile()
    return nc


_nc_cache = None


def _get_nc():
    global _nc_cache
    if _nc_cache is None:
        _nc_cache = _build()
    return _nc_cache


def prepare_inputs(inputs):
    """Host-side prep shared by kernel() and the timing harness.

    Returns (in_maps, b0): per-core device inputs and the bias to add on
    the host after the gather.
    """
    import ml_dtypes

    v = np.ascontiguousarray(np.asarray(inputs["v"], dtype=np.float32))
    Wv = np.asarray(inputs["Wv"], dtype=np.float32)
    W0 = np.asarray(inputs["W0"], dtype=np.float32)
    b0 = np.asarray(inputs["b0"], dtype=np.float32)

    # Fuse the two linear layers on the host: WcT = (W0 @ Wcat)^T  [C_in, C_out]
    Wc = W0 @ Wv.reshape(H * D, C)
    # wp[p, k, j] = WcT[k*128+p, j]
    wp = np.ascontiguousarray(
        Wc.T.reshape(KT, P, C).transpose(1, 0, 2).astype(ml_dtypes.bfloat16)
    )
    # vp[b][p, m, k, t] = v[b, m*128+t, k*128+p]
    vp = np.ascontiguousarray(
        v.reshape(B, MT, P, KT, P).transpose(0, 4, 1, 3, 2).astype(ml_dtypes.bfloat16)
    )
    return [{"vp": vp[i], "wp": wp} for i in range(B)], b0


def kernel(**inputs):
    in_maps, b0 = prepare_inputs(inputs)
    nc = _get_nc()
    res = run_bass_kernel_spmd(nc, in_maps, core_ids=list(range(B)))
    out = np.stack([res.results[i]["out"] for i in range(B)], axis=0)
    return out.astype(np.float32) + b0


# revision 33
# speedup vs baseline: 1.0880x; 1.0091x over previous
"""Trainium2 Bass kernel for nn_MultiHeadAttention_61546881352366.

The reference module's observable output is NOT attention: the attention
result is dead code in the original torch module.  The output is

    out = fc0(concat_h(v @ Wv_h^T)) = (v @ Wcat^T) @ W0^T + b0

with Wcat = Wv.reshape(H*D, C).  Two chained linear maps fuse into one:

    out = v @ (W0 @ Wcat)^T + b0 = v @ WcT + b0,   WcT = (W0 @ Wcat)^T

so the device work is a single [B*T, C] @ [C, C] matmul.  k and q are
unused.  The bias add and the final upcast to fp32 happen on the host.

Sharding: data-parallel over batch (B == 8 == n_cores); each core computes
one batch element's [2048, 1024] @ [1024, 1024] product in bf16 (fp32 PSUM
accumulate; rel err ~3e-3 incl. the bf16 output rounding).  Weights are
replicated (2 MiB/core).

Device kernel (per core):
  - inputs laid out on the host so every DMA moves contiguous
    per-partition lines:
      vp [128, 16, 8, 128] bf16   vp[p,m,k,t] = v[m*128+t, k*128+p]
      wp [128, 8, 1024]   bf16    wp[p,k,j]   = WcT[k*128+p, j]
  - 256 matmuls of [128x128] @ [128x512] bf16 at the warm PE rate
    (~216 ns each)
  - input chunks round-robined across the sync + scalar HW-DGE queues
    in the order the fill phase consumes them; the fill matmul order is
    matched to chunk arrival so the PE never starves while inputs
    stream in; warmup matmuls keep the PE busy from the end of the
    engine preamble until the first chunks land, so the HAM clock gate
    unthrottles to 2.4 GHz before the first real matmul
  - per (row tile, column half): one [128,512] PSUM bank, one vector
    copy fp32->bf16 into SBUF, one 128 KiB output DMA on the scalar
    queue; j-outer order so each bank drains while the next accumulates
  - the last row tile's second half runs as two 256-wide accumulation
    groups (N=256 matmul gaps are half of N=512, so this is free) so
    only a quarter-width drain trails the very last matmul
"""

import numpy as np

import concourse.bacc as bacc
import concourse.mybir as mybir
from concourse.tile import TileContext
from concourse.bass_utils import run_bass_kernel_spmd

B, T, C = 8, 2048, 1024
H, D = 16, 64
P = 128
KT = C // P   # 8 contraction tiles
MT = T // P   # 16 row tiles per core
NF = 512      # matmul moving free dim (= one PSUM bank of fp32)
NJ = C // NF  # 2 output column halves

_FP32 = mybir.dt.float32
_BF16 = mybir.dt.bfloat16

N_WARMUP = 11  # dummy matmuls sized to keep the PE busy from the end of the
               # engine preamble (~7.8us) until the first input chunks land
               # (~12.6us) -- the HAM clock gate then unthrottles BEFORE the
               # first real matmul, so the whole real stream runs at 2.4 GHz


def _build():
    nc = bacc.Bacc()
    vp = nc.dram_tensor("vp", [P, MT, KT, P], _BF16, kind="ExternalInput")
    wp = nc.dram_tensor("wp", [P, KT, C], _BF16, kind="ExternalInput")
    out = nc.dram_tensor("out", [T, C], _BF16, kind="ExternalOutput")

    with TileContext(nc) as tc:
        with (
            tc.tile_pool(name="wpool", bufs=1) as wpool,
            tc.tile_pool(name="vpool", bufs=1) as vpool,
            tc.tile_pool(name="spool", bufs=1) as spool,
            tc.tile_pool(name="opool", bufs=8) as opool,
            tc.tile_pool(name="pspool", bufs=8, space="PSUM") as pspool,
        ):
            # PE warmup: dependency-free matmuls on a memset tile so the HAM
            # clock gate ramps to 2.4 GHz while the first DMAs are in flight.
            # gpsimd does the memset -- it is free earliest in the preamble.
            scratch = spool.tile([P, NF], _BF16, name="scratch", tag="scratch")
            nc.gpsimd.memset(scratch, 0.0)
            ps_w = pspool.tile([P, NF], _FP32, name="ps_w", tag="ps")
            for _ in range(N_WARMUP):
                nc.tensor.matmul(
                    ps_w, lhsT=scratch[:, :P], rhs=scratch, start=True, stop=True
                )

            # Input DMAs across the two HW-DGE queues (sync, scalar), ordered
            # so the fill phase's dependencies land in consumption order.
            # The first chunks are small so the very first matmuls' inputs
            # arrive as early as possible.
            w_sb = {}           # (k, j) -> (tile, row index, column offset)
            v_sb = [None] * MT  # per m-tile: (tile, index within group)

            def dma_w(ks, eng, jhalf=None):
                # jhalf None: full C columns; 0/1: one 512-wide half
                lo = 0 if jhalf is None else jhalf * NF
                hi = C if jhalf is None else (jhalf + 1) * NF
                w_c = wpool.tile(
                    [P, len(ks), hi - lo],
                    _BF16,
                    name=f"w_{ks[0]}_{lo}",
                    tag=f"w_{ks[0]}_{lo}",
                )
                eng.dma_start(out=w_c, in_=wp[:, ks[0] : ks[-1] + 1, lo:hi])
                for i, k in enumerate(ks):
                    for j in range(NJ):
                        if jhalf is None:
                            w_sb[(k, j)] = (w_c, i, j * NF)
                        elif j == jhalf:
                            w_sb[(k, j)] = (w_c, i, 0)

            def dma_v(lo, hi, eng):
                vt = vpool.tile(
                    [P, hi - lo, KT, P], _BF16, name=f"v_{lo}", tag=f"v_{lo}"
                )
                eng.dma_start(out=vt, in_=vp[:, lo:hi, :, :])
                for m in range(lo, hi):
                    v_sb[m] = (vt, m - lo)

            # Round-robin the chunks across the two HW-DGE queues (sync,
            # scalar) in consumption order -- which queue starts first /
            # runs faster varies run to run, so neither queue may own the
            # whole critical path.
            chunks = [
                lambda e: dma_w([0, 1], e),    # w k01     (512K)
                lambda e: dma_v(0, 1, e),      # v m0      (256K)
                lambda e: dma_v(1, 2, e),      # v m1      (256K)
                lambda e: dma_w([2, 3], e),    # w k23     (512K)
                lambda e: dma_v(2, 3, e),      # v m2      (256K)
                lambda e: dma_v(3, 4, e),      # v m3      (256K)
                lambda e: dma_w([4, 5], e),    # w k45     (512K)
                lambda e: dma_w([6, 7], e),    # w k67     (512K)
                lambda e: dma_v(4, 8, e),      # v m4-7    (1M)
                lambda e: dma_v(8, 12, e),     # v m8-11   (1M)
                lambda e: dma_v(12, 16, e),    # v m12-15  (1M)
            ]
            for i, ch in enumerate(chunks):
                ch(nc.sync if i % 2 == 0 else nc.scalar)

            def mm(ps_mj, m, k, j, colo=0, width=NF):
                vt, s = v_sb[m]
                wt, i, off = w_sb[(k, j)]
                nc.tensor.matmul(
                    ps_mj,
                    lhsT=vt[:, s, k, :],
                    rhs=wt[:, i, off + colo : off + colo + width],
                    start=(k == 0),
                    stop=(k == KT - 1),
                )

            ps = {}
            ob = {}

            def new_ps(m):
                ps[m] = [
                    pspool.tile([P, NF], _FP32, name=f"ps_{m}_{j}", tag="ps")
                    for j in range(NJ)
                ]

            def drain(m, j):
                if m not in ob:
                    ob[m] = opool.tile([P, C], _BF16, name=f"ob_{m}", tag="ob")
                rows = slice(m * P, (m + 1) * P)
                sl = slice(j * NF, (j + 1) * NF)
                nc.vector.tensor_copy(ob[m][:, sl], ps[m][j])
                nc.scalar.dma_start(out=out[rows, sl], in_=ob[m][:, sl])

            def drain_part(m, base, width, src):
                rows = slice(m * P, (m + 1) * P)
                sl = slice(base, base + width)
                nc.vector.tensor_copy(ob[m][:, sl], src)
                nc.scalar.dma_start(out=out[rows, sl], in_=ob[m][:, sl])

            # Fill phase (m0-3): blocks ordered to match DMA chunk arrival.
            # Each entry is (k-list, m-list, j-list), emitted m/k/j nested.
            for m in range(4):
                new_ps(m)
            fill = [
                ([0, 1], [0], [0, 1]),
                ([0, 1], [1], [0, 1]),
                ([2, 3], [0, 1], [0, 1]),
                ([0, 1], [2], [0, 1]),
                ([0, 1], [3], [0, 1]),
                ([2, 3], [2, 3], [0, 1]),
                ([4, 5], [0, 1, 2, 3], [0, 1]),
            ]
            for ks, ms, js in fill:
                for m in ms:
                    for k in ks:
                        for j in js:
                            mm(ps[m][j], m, k, j)
            # last fill chunk: k6-7, j-outer per m so each bank drains early
            for m in range(4):
                for j in range(NJ):
                    for k in (6, 7):
                        mm(ps[m][j], m, k, j)
                    drain(m, j)

            # Steady phase (m4-15): j-outer, k-inner; each column half
            # drains as soon as its 8 matmuls finish.
            for m in range(4, MT - 1):
                new_ps(m)
                for j in range(NJ):
                    for k in range(KT):
                        mm(ps[m][j], m, k, j)
                    drain(m, j)

            # Last row tile: j1 runs as two 256-wide accumulation groups
            # (same total matmul time -- N=256 gaps are half of N=512) so
            # only a quarter-width drain trails the very last matmul.
            m = MT - 1
            new_ps(m)
            ob[m] = opool.tile([P, C], _BF16, name=f"ob_{m}", tag="ob")
            for k in range(KT):
                mm(ps[m][0], m, k, 0)
            drain(m, 0)
            psq = [
                pspool.tile([P, NF // 2], _FP32, name=f"ps_{m}_1{g}", tag="ps")
                for g in range(2)
            ]
            for g in range(2):
                for k in range(KT):
                    mm(psq[g], m, k, 1, colo=g * (NF // 2), width=NF // 2)
                drain_part(m, NF + g * (NF // 2), NF // 2, psq[g])
    nc.compile()
    return nc


_nc_cache = None


def _get_nc():
    global _nc_cache
    if _nc_cache is None:
        _nc_cache = _build()
    return _nc_cache


def prepare_inputs(inputs):
    """Host-side prep shared by kernel() and the timing harness.

    Returns (in_maps, b0): per-core device inputs and the bias to add on
    the host after the gather.
    """
    import ml_dtypes

    v = np.ascontiguousarray(np.asarray(inputs["v"], dtype=np.float32))
    Wv = np.asarray(inputs["Wv"], dtype=np.float32)
    W0 = np.asarray(inputs["W0"], dtype=np.float32)
    b0 = np.asarray(inputs["b0"], dtype=np.float32)

    # Fuse the two linear layers on the host: WcT = (W0 @ Wcat)^T  [C_in, C_out]
    Wc = W0 @ Wv.reshape(H * D, C)
    # wp[p, k, j] = WcT[k*128+p, j]
    wp = np.ascontiguousarray(
        Wc.T.reshape(KT, P, C).transpose(1, 0, 2).astype(ml_dtypes.bfloat16)
    )
    # vp[b][p, m, k, t] = v[b, m*128+t, k*128+p]
    vp = np.ascontiguousarray(
        v.reshape(B, MT, P, KT, P).transpose(0, 4, 1, 3, 2).astype(ml_dtypes.bfloat16)
    )
    return [{"vp": vp[i], "wp": wp} for i in range(B)], b0


def kernel(**inputs):
    in_maps, b0 = prepare_inputs(inputs)
    nc = _get_nc()
    res = run_bass_kernel_spmd(nc, in_maps, core_ids=list(range(B)))
    out = np.stack([res.results[i]["out"] for i in range(B)], axis=0)
    return out.astype(np.float32) + b0
